# revision 23
# baseline (speedup 1.0000x reference)
"""Trainium2 Bass kernel for nn_Decoder (mask-multiply + dense [512,16] + overlap-and-add).

Full-input contract: kernel(**inputs) takes the complete tensors, shards
batch-wise across 8 NeuronCores (2 batches per core, both speakers on-core),
runs one SPMD Bass program, and gathers the full [16, 2, 32696] output.

Per-core algorithm (b = 2 batches, frame = 4086, basis = 512, spk = 2, L = 16):
  1. DMA inputs[b, f0:f0+512, :]  -> SBUF [128, 4, 512]   (f on partitions)
     DMA estmask flattened        -> SBUF [128, 4, 1024]  (free dim = 2c+s)
  2. DVE: de-interleave + mask-multiply -> xx[128, 4, 1024] (free = s*512+c)
  3. PE transpose 128x128 blocks -> PSUM -> ACT copy -> xxT[128, 8, 512]
     (c2 = s*512+c on partitions, f on free dim)
  4. PE matmul per speaker: yyT[16, Fb] += W[ck].T @ xxT[ck]  (4 c-chunks)
  5. ACT copy yyT -> SBUF staging st[16, Fb]; SBUF->SBUF DMA shifts the high
     taps st[8:16] into row buffer zb[8, nseg] at column f0+1 (DMA is the only
     engine free of partition-base constraints)
  6. DVE overlap-add per block: z[j, k] = st[j, k] + zb[j, k]  (zb col 0 zero)
  7. DMA z[8, Fb] -> out[b, s] viewed as [4087, 8] (n = 8k + j); one tail
     column k = 4086 comes straight from zb after the last block
"""

import sys

for _p in ("/opt/trn_rl_repo", "/root/.axon_site/_ro/trn_rl_repo"):
    if _p not in sys.path:
        sys.path.append(_p)

import numpy as np

# Problem constants (hardcoded per contract; kernel.py may not read spec.json).
BS = 16
FRAME = 4086
BASIS = 512
SPK = 2
L = 16
STEP = L // 2
OUT_LEN = (FRAME - 1) * STEP + L  # 32696
NSEG = OUT_LEN // STEP  # 4087 == FRAME + 1
N_CORES = 8
B_PER_CORE = BS // N_CORES  # 2


def _split_excess_waits(nc, max_waits=1):
    """This toolchain's walrus rejects >1 semaphore wait per instruction
    ("Too many sync wait commands"), including on Tile's own kernel-tail
    drain. Move excess waits onto standalone EventSemaphore instructions
    inserted just before the owner — the same-engine sequencer executes them
    in order, which is semantically identical."""
    import concourse.mybir as mybir

    n = 0
    for fn in nc.m.functions:
        for blk in fn.blocks:
            out = []
            for inst in list(blk.instructions):
                si = inst.sync_info
                waits = list(si.on_wait) if si is not None else []
                if len(waits) > max_waits:
                    for w in waits[max_waits:]:
                        n += 1
                        out.append(
                            mybir.InstEventSemaphore(
                                name=f"WSPLIT-{n}",
                                engine=inst.engine,
                                ins=[],
                                outs=[],
                                sync_info=mybir.SyncInfo(on_wait=[w], on_update=[]),
                            )
                        )
                    inst.sync_info = mybir.SyncInfo(
                        on_wait=waits[:max_waits], on_update=list(si.on_update)
                    )
                out.append(inst)
            blk.instructions = out
    return n


def build_decoder_program(B, frame, basis, spk, Lk, fb=512, split_waits=True):
    """Build the per-core Bass program. All shapes parameterized so the same
    builder can be validated in CoreSim at small sizes."""
    import concourse.bass as bass
    import concourse.mybir as mybir
    import concourse.tile as tile
    from concourse.bass import ds
    from contextlib import ExitStack

    f32 = mybir.dt.float32
    step = Lk // 2
    nseg = frame + 1
    out_len = (frame - 1) * step + Lk
    assert out_len == nseg * step
    dbl = basis * spk
    KC = basis // 128  # c-chunks per speaker
    NCH = dbl // 128  # c2 chunks total
    nblocks = (frame + fb - 1) // fb
    nsub_max = fb // 128

    nc = bass.Bass()
    inputs_d = nc.dram_tensor("inputs", [B, frame, basis], f32, kind="ExternalInput")
    mask_d = nc.dram_tensor("estmask", [B, frame, dbl], f32, kind="ExternalInput")
    w_d = nc.dram_tensor("w", [basis, Lk], f32, kind="ExternalInput")
    ident_d = nc.dram_tensor("ident", [128, 128], f32, kind="ExternalInput")
    out_d = nc.dram_tensor("out", [B, spk, out_len], f32, kind="ExternalOutput")

    with ExitStack() as ctx:
        tc = ctx.enter_context(tile.TileContext(nc))
        singles = ctx.enter_context(tc.tile_pool(name="singles", bufs=1))
        in_pool = ctx.enter_context(tc.tile_pool(name="inp", bufs=2))
        mk_pool = ctx.enter_context(tc.tile_pool(name="mk", bufs=2))
        xx_pool = ctx.enter_context(tc.tile_pool(name="xx", bufs=2))
        xxt_pool = ctx.enter_context(tc.tile_pool(name="xxt", bufs=2))
        yrow_pool = ctx.enter_context(tc.tile_pool(name="yrow", bufs=1))
        st_pool = ctx.enter_context(tc.tile_pool(name="st", bufs=4))
        z_pool = ctx.enter_context(tc.tile_pool(name="z", bufs=4))
        tp_psum = ctx.enter_context(tc.tile_pool(name="tp_psum", bufs=3, space="PSUM"))
        yy_psum = ctx.enter_context(tc.tile_pool(name="yy_psum", bufs=4, space="PSUM"))

        w_sb = singles.tile([128, KC, Lk], f32)
        nc.sync.dma_start(out=w_sb, in_=w_d[:].rearrange("(k p) l -> p k l", p=128))
        ident = singles.tile([128, 128], f32)
        nc.sync.dma_start(out=ident, in_=ident_d[:])

        for b in range(B):
            # zb[s][j, k] = y_s[k-1, j+step]  (zero at k = 0)
            zb = [
                yrow_pool.tile([step, nseg], f32, tag=f"zb{s}", name=f"zb{s}")
                for s in range(spk)
            ]
            for s in range(spk):
                nc.vector.memset(zb[s][:, 0:1], 0.0)
            for ib in range(nblocks):
                f0 = ib * fb
                Fb = min(fb, frame - f0)
                nsub = (Fb + 127) // 128
                in_t = in_pool.tile([128, nsub_max, basis], f32, tag="in_t")
                mk_t = mk_pool.tile([128, nsub_max, dbl], f32, tag="mk_t")
                if Fb == nsub_max * 128:
                    nc.sync.dma_start(
                        out=in_t,
                        in_=inputs_d[b, f0 : f0 + Fb, :].rearrange("(a p) c -> p a c", p=128),
                    )
                    nc.sync.dma_start(
                        out=mk_t,
                        in_=mask_d[b, f0 : f0 + Fb, :].rearrange("(a p) c -> p a c", p=128),
                    )
                else:
                    for a in range(nsub):
                        ps = min(128, Fb - a * 128)
                        nc.sync.dma_start(
                            out=in_t[:ps, a, :],
                            in_=inputs_d[b, f0 + a * 128 : f0 + a * 128 + ps, :],
                        )
                        nc.sync.dma_start(
                            out=mk_t[:ps, a, :],
                            in_=mask_d[b, f0 + a * 128 : f0 + a * 128 + ps, :],
                        )
                xx_t = xx_pool.tile([128, nsub_max, dbl], f32, tag="xx_t")
                for a in range(nsub):
                    ps = min(128, Fb - a * 128)
                    mk_r = mk_t[:ps, a, :].rearrange("p (c two) -> p two c", two=2)
                    for s in range(spk):
                        nc.vector.tensor_mul(
                            xx_t[:ps, a, ds(s * basis, basis)],
                            in_t[:ps, a, :],
                            mk_r[:, s, :],
                        )
                xxT_t = xxt_pool.tile([128, NCH, fb], f32, tag="xxT_t")
                for k in range(NCH):
                    ps_t = tp_psum.tile([128, fb], f32, tag="ps_t")
                    for a in range(nsub):
                        ps = min(128, Fb - a * 128)
                        nc.tensor.transpose(
                            ps_t[:, ds(a * 128, ps)],
                            xx_t[:ps, a, ds(k * 128, 128)],
                            ident[:ps, :ps],
                        )
                    nc.scalar.copy(out=xxT_t[:, k, :Fb], in_=ps_t[:, :Fb])
                for s in range(spk):
                    yy_t = yy_psum.tile([Lk, fb], f32, tag="yy_t")
                    for kc in range(KC):
                        k = s * KC + kc
                        nc.tensor.matmul(
                            yy_t[:, :Fb],
                            w_sb[:, kc, :],
                            xxT_t[:, k, :Fb],
                            start=(kc == 0),
                            stop=(kc == KC - 1),
                        )
                    st_t = st_pool.tile([Lk, fb], f32, tag="st_t")
                    nc.scalar.copy(out=st_t[:, :Fb], in_=yy_t[:, :Fb])
                    # partition-shift the high taps into the row buffer
                    nc.sync.dma_start(
                        out=zb[s][:, f0 + 1 : f0 + 1 + Fb],
                        in_=st_t[step:Lk, :Fb],
                    )
                    z_t = z_pool.tile([step, fb], f32, tag="z_t")
                    nc.vector.tensor_add(
                        z_t[:, :Fb], st_t[0:step, :Fb], zb[s][:, f0 : f0 + Fb]
                    )
                    nc.sync.dma_start(
                        out=out_d[b, s, :].rearrange("(k j) -> j k", j=step)[
                            :, f0 : f0 + Fb
                        ],
                        in_=z_t[:, :Fb],
                    )
            for s in range(spk):
                # tail segment k = frame: z = y[frame-1, j+step] only
                nc.sync.dma_start(
                    out=out_d[b, s, :].rearrange("(k j) -> j k", j=step)[
                        :, nseg - 1 : nseg
                    ],
                    in_=zb[s][:, nseg - 1 : nseg],
                )
    if split_waits:
        _split_excess_waits(nc)
    return nc


_PROGRAM_CACHE = {}


def _get_program():
    key = (B_PER_CORE, FRAME, BASIS, SPK, L)
    if key not in _PROGRAM_CACHE:
        _PROGRAM_CACHE[key] = build_decoder_program(*key)
    return _PROGRAM_CACHE[key]


def prepare_in_maps(inputs, estmask, W):
    """Shard the full inputs into per-core input maps."""
    inputs = np.ascontiguousarray(np.asarray(inputs, dtype=np.float32))
    estmask = np.ascontiguousarray(np.asarray(estmask, dtype=np.float32))
    W = np.ascontiguousarray(np.asarray(W, dtype=np.float32))
    mflat = estmask.reshape(BS, FRAME, BASIS * SPK)
    ident = np.eye(128, dtype=np.float32)

    in_maps = []
    for c in range(N_CORES):
        b0 = c * B_PER_CORE
        in_maps.append(
            {
                "inputs": inputs[b0 : b0 + B_PER_CORE],
                "estmask": mflat[b0 : b0 + B_PER_CORE],
                "w": W,
                "ident": ident,
            }
        )
    return in_maps


def run(inputs, estmask, W, trace=False):
    """Shard across 8 cores, run SPMD, gather. Returns (out, BassKernelResults)."""
    from concourse.bass_utils import run_bass_kernel_spmd

    nc = _get_program()
    in_maps = prepare_in_maps(inputs, estmask, W)
    res = run_bass_kernel_spmd(nc, in_maps, core_ids=list(range(N_CORES)), trace=trace)
    out = np.empty((BS, SPK, OUT_LEN), dtype=np.float32)
    for c in range(N_CORES):
        out[c * B_PER_CORE : (c + 1) * B_PER_CORE] = res.results[c]["out"]
    return out, res


def kernel(inputs, estmask, W, kernel_size_enc=None, speech_length=None):
    out, _ = run(inputs, estmask, W, trace=False)
    return out
